# revision 7
# baseline (speedup 1.0000x reference)
"""Trainium2 Bass kernel for nn_CompSAE (topk_masking, memory-bound).

Math (after host-side folding of the seq_len-1 attention + biases):
    f  = relu(x @ W1 + b1_eff)            # [N, 256],  W1 = W_enc_f
    c  = relu(f @ W2 + b2)                # [N, 128],  W2 = W_enc_c
    bn = relu(c @ Wb + bb)                # [N, 32],   Wb = W_v.T @ W_out.T @ W_bottleneck
    y  = bn @ W_dec + f @ W_res + b_dec   # [N, 2048]

Sharding: pure data-parallel over the token axis N=131072 across 8 cores
(16384 tokens/core). All weights replicated.

The host pre-transposes + fp16-casts x (so the contraction dim lands on SBUF
partitions with plain contiguous DMAs), and all matmuls run in fp16 with fp32
PSUM accumulation (~4e-4 scale-relative error). Activations stay feature-major
(fT/cT/bnT = [feature, token]) so every matmul's stationary operand is either a
natural-layout weight chunk or a feature-major activation; the final decode
matmul then lands token-major, matching the contiguous fp32 output DMA.
b_dec rides along as a ones-row in the bnT stationary (Wdec augmented row).
"""

import os
import numpy as np

N_TOK, D_IN, D_F, D_C, K_BN = 131072, 2048, 256, 128, 32
N_CORES = 8
SHARD = N_TOK // N_CORES          # 16384 tokens per core
TOK = 512                         # supertile tokens
N_SUPER = SHARD // TOK            # 32 supertiles
KC = D_IN // 128                  # 16 contraction chunks for mm1

_CACHE = {}


def _build_nc():
    import concourse.tile as tile
    from concourse import bacc, mybir
    from concourse.bass import ts

    f32 = mybir.dt.float32
    f16 = mybir.dt.float16
    Relu = mybir.ActivationFunctionType.Relu

    nc = bacc.Bacc(None, target_bir_lowering=False)

    xT_d = nc.dram_tensor("xT", [D_IN, SHARD], f16, kind="ExternalInput")
    w1_d = nc.dram_tensor("w1", [KC, 128, D_F], f16, kind="ExternalInput")
    w2_d = nc.dram_tensor("w2", [2, 128, D_C], f16, kind="ExternalInput")
    # wb zero-padded to [128, 128] (cols 32..127 zero) and wdec_aug zero-padded
    # to [128, 2048] (rows 33..127 zero): uniform 128-row stationaries pipeline
    # on the PE; the K=33 form stalled ~250ns per matmul on reconfig.
    wb_d = nc.dram_tensor("wb", [D_C, 128], f16, kind="ExternalInput")
    wres_d = nc.dram_tensor("wres", [2, 128, D_IN], f16, kind="ExternalInput")
    wdec_d = nc.dram_tensor("wdec", [128, D_IN], f16, kind="ExternalInput")
    b1_d = nc.dram_tensor("b1", [128, 2], f32, kind="ExternalInput")
    b2_d = nc.dram_tensor("b2", [128, 1], f32, kind="ExternalInput")
    bb_d = nc.dram_tensor("bb", [K_BN, 1], f32, kind="ExternalInput")
    ones_d = nc.dram_tensor("ones", [1, TOK], f16, kind="ExternalInput")
    y_d = nc.dram_tensor("y", [SHARD, D_IN], f32, kind="ExternalOutput")

    with tile.TileContext(nc) as tc:
        with (
            tc.tile_pool(name="const", bufs=1) as const,
            tc.tile_pool(name="xtp", bufs=3) as xtp,
            tc.tile_pool(name="fp", bufs=2) as fp,
            tc.tile_pool(name="cp", bufs=2) as cp,
            tc.tile_pool(name="bnp", bufs=2) as bnp,
            tc.tile_pool(name="yp", bufs=2) as yp,
            tc.tile_pool(name="fps", bufs=2, space="PSUM") as fps,
            tc.tile_pool(name="sps", bufs=2, space="PSUM") as sps,
            tc.tile_pool(name="yps", bufs=3, space="PSUM") as yps,
        ):
            w1_sb = const.tile([128, KC, D_F], f16)
            nc.sync.dma_start(w1_sb[:], w1_d.rearrange("a p n -> p a n"))
            w2_sb = const.tile([128, 2, D_C], f16)
            nc.sync.dma_start(w2_sb[:], w2_d.rearrange("a p n -> p a n"))
            wb_sb = const.tile([D_C, 128], f16)
            nc.sync.dma_start(wb_sb[:], wb_d[:])
            wres_sb = const.tile([128, 2, D_IN], f16)
            nc.sync.dma_start(wres_sb[:], wres_d.rearrange("a p n -> p a n"))
            wdec_sb = const.tile([128, D_IN], f16)
            nc.sync.dma_start(wdec_sb[:], wdec_d[:])
            b1_sb = const.tile([128, 2], f32)
            nc.sync.dma_start(b1_sb[:], b1_d[:])
            b2_sb = const.tile([128, 1], f32)
            nc.sync.dma_start(b2_sb[:], b2_d[:])
            bb_sb = const.tile([K_BN, 1], f32)
            nc.sync.dma_start(bb_sb[:], bb_d[:])
            ones_sb = const.tile([1, TOK], f16)
            nc.sync.dma_start(ones_sb[:], ones_d[:])

            for t in range(N_SUPER):
                t0 = t * TOK
                xT = xtp.tile([128, KC, TOK], f16)
                nc.sync.dma_start(
                    xT[:], xT_d[:, t0:t0 + TOK].rearrange("(c p) t -> p c t", p=128)
                )

                # mm1: fT[df_chunk m] = sum_c W1[c,:,m*128:+128].T @ xT[c]
                fT = fp.tile([128, 2, TOK], f16)
                for m in range(2):
                    ps = fps.tile([128, TOK], f32)
                    for c in range(KC):
                        nc.tensor.matmul(
                            ps[:], w1_sb[:, c, ts(m, 128)], xT[:, c, :],
                            start=(c == 0), stop=(c == KC - 1),
                        )
                    nc.scalar.activation(fT[:, m, :], ps[:], Relu, bias=b1_sb[:, m:m + 1])

                # mm2: cT = sum_m W2[m].T @ fT[m]
                cps = sps.tile([128, TOK], f32, tag="small")
                for m in range(2):
                    nc.tensor.matmul(
                        cps[:], w2_sb[:, m, :], fT[:, m, :],
                        start=(m == 0), stop=(m == 1),
                    )
                cT = cp.tile([128, TOK], f16)
                nc.scalar.activation(cT[:], cps[:], Relu, bias=b2_sb[:])

                # mm3: bnT = Wb.T @ cT, ones row for b_dec, zero rows 33..127 so
                # the mm4 stationary is a uniform 128-row tile.
                bps = sps.tile([128, TOK], f32, tag="small")
                nc.tensor.matmul(bps[:], wb_sb[:], cT[:])
                bnT = bnp.tile([128, TOK], f16)
                nc.gpsimd.memset(bnT[:], 0.0)
                nc.scalar.activation(bnT[0:K_BN, :], bps[0:K_BN, :], Relu, bias=bb_sb[:])
                nc.vector.tensor_copy(out=bnT[K_BN:K_BN + 1, :], in_=ones_sb[:])

                # mm4: y[tok_half th, 512-chunk n] = fT.T@Wres + bnT_aug.T@Wdec_aug
                y_sb = yp.tile([128, TOK // 128, D_IN], f32)
                for th in range(TOK // 128):
                    for n in range(4):
                        ps = yps.tile([128, 512], f32)
                        nc.tensor.matmul(
                            ps[:], fT[:, 0, ts(th, 128)], wres_sb[:, 0, ts(n, 512)],
                            start=True, stop=False,
                        )
                        nc.tensor.matmul(
                            ps[:], fT[:, 1, ts(th, 128)], wres_sb[:, 1, ts(n, 512)],
                            start=False, stop=False,
                        )
                        nc.tensor.matmul(
                            ps[:], bnT[:, ts(th, 128)], wdec_sb[:, ts(n, 512)],
                            start=False, stop=True,
                        )
                        if (th * 4 + n) % 2 == 0:
                            nc.scalar.copy(out=y_sb[:, th, ts(n, 512)], in_=ps[:])
                        else:
                            nc.vector.tensor_copy(out=y_sb[:, th, ts(n, 512)], in_=ps[:])

                nc.scalar.dma_start(
                    y_d[t0:t0 + TOK, :].rearrange("(a p) d -> p a d", p=128), y_sb[:]
                )

    nc.compile()
    return nc


def _fold_weights(inputs):
    f64 = np.float64
    W1 = np.asarray(inputs["W_enc_f"], np.float32)
    W2 = np.asarray(inputs["W_enc_c"], np.float32)
    W_v = np.asarray(inputs["W_v"], f64)
    b_v = np.asarray(inputs["b_v"], f64)
    W_out = np.asarray(inputs["W_out"], f64)
    b_out = np.asarray(inputs["b_out"], f64)
    W_bn = np.asarray(inputs["W_bottleneck"], f64)
    W_dec = np.asarray(inputs["W_dec"], np.float32)
    b_dec = np.asarray(inputs["b_dec"], np.float32)
    W_res = np.asarray(inputs["W_res"], np.float32)
    b1_eff = (np.asarray(inputs["b_enc_f"], f64)
              - np.asarray(inputs["b_dec"], f64) @ np.asarray(inputs["W_enc_f"], f64))
    Wb = (W_v.T @ W_out.T) @ W_bn                      # [128, 32]
    bb = (b_v @ W_out.T + b_out) @ W_bn                # [32]
    wdec_aug = np.vstack([W_dec, b_dec[None, :]])      # [33, 2048]

    return {
        "w1": np.ascontiguousarray(W1.reshape(KC, 128, D_F).astype(np.float16)),
        "w2": np.ascontiguousarray(W2.reshape(2, 128, D_C).astype(np.float16)),
        "wb": np.ascontiguousarray(
            np.pad(Wb.astype(np.float16), ((0, 0), (0, 128 - K_BN)))),
        "wres": np.ascontiguousarray(W_res.reshape(2, 128, D_IN).astype(np.float16)),
        "wdec": np.ascontiguousarray(
            np.pad(wdec_aug.astype(np.float16), ((0, 128 - K_BN - 1), (0, 0)))),
        "b1": np.ascontiguousarray(b1_eff.astype(np.float32).reshape(2, 128).T),
        "b2": np.ascontiguousarray(np.asarray(inputs["b_enc_c"], np.float32).reshape(128, 1)),
        "bb": np.ascontiguousarray(bb.astype(np.float32).reshape(K_BN, 1)),
        "ones": np.ones((1, TOK), np.float16),
    }


def kernel(**inputs) -> np.ndarray:
    from concourse.bass_utils import run_bass_kernel_spmd

    if "nc" not in _CACHE:
        _CACHE["nc"] = _build_nc()
    nc = _CACHE["nc"]

    x = np.asarray(inputs["acts"], np.float32)
    weights = _fold_weights(inputs)

    in_maps = []
    for i in range(N_CORES):
        xT_i = np.ascontiguousarray(
            x[i * SHARD:(i + 1) * SHARD, :].T.astype(np.float16)
        )
        m = {"xT": xT_i}
        m.update(weights)
        in_maps.append(m)

    trace = bool(os.environ.get("BASS_KERNEL_TRACE"))
    res = run_bass_kernel_spmd(
        nc, in_maps, core_ids=list(range(N_CORES)), trace=trace,
    )
    _CACHE["last_result"] = res
    return np.concatenate([res.results[i]["y"] for i in range(N_CORES)], axis=0)
